# revision 38
# baseline (speedup 1.0000x reference)
"""LSTMCell forward on 8 Trainium2 NeuronCores (Bass/Tile, SPMD data-parallel).

Strategy (v5: transposed-z, fp16, weight-stationary reuse, engine-split DMA):
  - Shard the batch (32768) across 8 cores: 4096 rows each.
  - Compute z TRANSPOSED: z^T[n, b] = sum_k W[k, n] * xh[b, k].  Gate dim n
    (2048) on PSUM partitions in 16 chunks of 128; batch is the moving dim.
    lhsT = weight chunk [128k, 128n] (stationary), rhs = xh^T [128k, 512b].
  - All 2-byte tensors fp16 (same PE/DVE rate as bf16, ~8x less rounding:
    rel err ~1e-3 vs the 2e-2 gate), fp32 PSUM accumulate.
  - Weight-stationary reuse: per (dc, gate, kc) the same lhsT feeds NCH=4
    adjacent matmuls (4 batch chunks of 512).  tile_legalize splits every
    matmul into LDWEIGHTS+MATMUL with no dedup; a post-pass deletes the
    redundant LDWEIGHTS -> 256 real weight loads instead of 1024.
  - Epilogue: ACT evacuates PSUM fused with the per-partition bias add
    (bias varies along partitions in this layout), DVE does the gate math
    in fp16, outputs written transposed (host un-transposes + upcasts).
  - DMA engine split: inputs prefetch on the SP HWDGE ring (hoisted to body
    start, never blocked by compute-gated entries), outputs trigger from the
    ACT HWDGE ring delayed by one d-chunk so their waits are pre-satisfied.
"""
import sys
from contextlib import nullcontext

if "/opt/trn_rl_repo" not in sys.path:
    sys.path.insert(0, "/opt/trn_rl_repo")

import numpy as np
import concourse.bass as bass
import concourse.mybir as mybir
from concourse.tile import TileContext
from concourse.bass_utils import run_bass_kernel_spmd

F32 = mybir.dt.float32
F16 = mybir.dt.float16
AF = mybir.ActivationFunctionType
NP_F16 = np.float16

N_CORES = 8
P = 128
DH = 512
DH4 = 4 * DH            # 2048
K = 1024                # concat(x, h) contraction dim
KT = K // P             # 8 k-chunks
NDC = DH // P           # 4 d-chunks per gate
B_FULL = 32768
B_CORE = B_FULL // N_CORES   # 4096
CHUNK = 512                  # moving-dim width per matmul
NCH = 4                      # batch chunks per macro sharing one stationary
MACRO = CHUNK * NCH          # 2048
NMACRO = B_CORE // MACRO     # 2
DEDUP_LDWEIGHTS = True
COALESCE_SEM = True


def fanout_multi_waits(nc):
    """This walrus build rejects >1 sync wait per instruction: fan extra
    waits out onto single-wait NoOps on the same (in-order) engine."""
    n = 0
    for f in nc.m.functions:
        for bb in f.blocks:
            new = []
            for inst in bb.instructions:
                si = inst.sync_info
                waits = list(si.on_wait) if si and si.on_wait else []
                if len(waits) > 1:
                    for w in waits[:-1]:
                        nop = mybir.InstNoOp(name=f"waitfan_{n}", ins=[], outs=[])
                        n += 1
                        nop.engine = inst.engine
                        nop.sync_info = mybir.SyncInfo(on_wait=[w], on_update=[])
                        new.append(nop)
                    si.on_wait = [waits[-1]]
                new.append(inst)
            bb.instructions = new
    return n


def _ap_sig(arg):
    """Structural signature of a lowered AP argument (memory location +
    offset + access pattern + dtype)."""
    try:
        return repr(arg)
    except Exception:
        return None


def dedup_ldweights(nc):
    """Drop an InstLdweights whose weights AP is identical to the previous
    one on the PE engine (with only matmuls/noops in between).  The PE array
    keeps the stationary across matmuls, so the reload is redundant
    (LDWEIGHTS + MATMUL xN).  Sync-free redundant loads are deleted outright
    (a NoOp would still occupy the PE sequencer); ones carrying sync become
    NoOps."""
    n = 0
    for f in nc.m.functions:
        for bb in f.blocks:
            last_sig = None
            keep = []
            for inst in bb.instructions:
                if inst.engine != mybir.EngineType.PE:
                    keep.append(inst)
                    continue
                ty = type(inst).__name__
                if ty == "InstLdweights":
                    sig = _ap_sig(inst.ins[0]) if inst.ins else None
                    if sig is not None and sig == last_sig:
                        si = inst.sync_info
                        has_sync = si is not None and (
                            len(si.on_wait or []) or len(si.on_update or [])
                        )
                        n += 1
                        if has_sync:
                            nop = mybir.InstNoOp(
                                name=f"lwdedup_{n}", ins=[], outs=[]
                            )
                            nop.engine = inst.engine
                            nop.sync_info = inst.sync_info
                            keep.append(nop)
                        continue
                    last_sig = sig
                elif ty not in ("InstMatmult", "InstNoOp"):
                    last_sig = None
                keep.append(inst)
            bb.instructions = keep
    return n


def coalesce_mm_sem_updates(nc):
    """Within each run of consecutive wait-free PE matmuls (ldweights/noops
    transparent), move the per-matmul sem-inc updates onto the run's last
    matmul with the summed value.  Runs break at any PE instruction that
    waits (so waiter timing at group boundaries is preserved and no
    wait-for-own-inc deadlock is possible)."""
    n_removed = 0

    def flush(run):
        nonlocal n_removed
        # run: list of matmuls with single sem-inc updates on the same sem
        if len(run) < 2:
            return
        total = len(run)
        last = run[-1]
        old = last.sync_info.on_update[0]
        # sem-add-imm carries a value ("sem-inc" is always +1).  Putting the
        # summed update on the run's LAST matmul is visibility-safe: PE psum
        # drains are FIFO, so by the time the last matmul's deferred update
        # fires, every earlier matmul in the run has drained.
        coalesced = mybir.SyncUpdate(
            sync_type="semaphore",
            id=old.id,
            update_mode="sem-add-imm",
            update_value=total,
            ant_name=old.ant_name,
        )
        for inst in run[:-1]:
            inst.sync_info = mybir.SyncInfo(
                on_wait=list(inst.sync_info.on_wait or []), on_update=[]
            )
        last.sync_info = mybir.SyncInfo(
            on_wait=list(last.sync_info.on_wait or []), on_update=[coalesced]
        )
        n_removed += total - 1

    for f in nc.m.functions:
        for bb in f.blocks:
            run = []
            run_sem = None
            for inst in bb.instructions:
                if inst.engine != mybir.EngineType.PE:
                    continue
                ty = type(inst).__name__
                if ty in ("InstLdweights", "InstNoOp"):
                    si = inst.sync_info
                    if si is not None and (si.on_wait or si.on_update):
                        flush(run)
                        run, run_sem = [], None
                    continue
                if ty != "InstMatmult":
                    flush(run)
                    run, run_sem = [], None
                    continue
                si = inst.sync_info
                waits = list(si.on_wait) if si and si.on_wait else []
                ups = list(si.on_update) if si and si.on_update else []
                simple = (
                    len(ups) == 1
                    and str(ups[0].update_mode) == "sem-inc"
                    and ups[0].update_value == 1
                )
                if waits or not simple:
                    flush(run)
                    run, run_sem = [], None
                    if simple and not waits:
                        run, run_sem = [inst], ups[0].id
                    continue
                if run and ups[0].id == run_sem:
                    run.append(inst)
                else:
                    flush(run)
                    run, run_sem = [inst], ups[0].id
            flush(run)
    return n_removed


def build_nc(loop_n=None, skip_out_dma=False, skip_epilogue=False,
             skip_in_dma=False, probe=None):
    """Build the per-core program. loop_n wraps the body in a device-side
    For_i repeat (timing probe; outputs unchanged since the body is
    idempotent).  skip_out_dma / skip_epilogue are ablation probes for
    bench-only decomposition (outputs are garbage when set)."""
    nc = bass.Bass()
    # Pre-arranged on host: xhT[p, kc, b] = concat(x,h)[b, kc*128+p]
    xhT = nc.dram_tensor("xhT", [P, KT, B_CORE], F16, kind="ExternalInput")
    # CT[p, dc, b] = C[b, dc*128+p]
    CT = nc.dram_tensor("CT", [P, NDC, B_CORE], F16, kind="ExternalInput")
    # W[p, kc, n] = vstack(Wx, Wh)[kc*128+p, n]
    W = nc.dram_tensor("W", [P, KT, DH4], F16, kind="ExternalInput")
    # bias[p, nc] = (bx+bh)[nc*128+p]
    bias = nc.dram_tensor("bias", [P, DH4 // P], F32, kind="ExternalInput")
    CnT = nc.dram_tensor("CnT", [P, NDC, B_CORE], F16, kind="ExternalOutput")
    HnT = nc.dram_tensor("HnT", [P, NDC, B_CORE], F16, kind="ExternalOutput")

    with TileContext(nc) as tc:
        with (
            tc.tile_pool(name="const", bufs=1) as const,
            tc.tile_pool(name="io", bufs=2) as io,
            tc.tile_pool(name="gates", bufs=2) as gates,
            tc.tile_pool(name="work", bufs=3) as work,
            tc.tile_pool(name="psum", bufs=2, space=bass.MemorySpace.PSUM) as psum,
        ):
            w_t = const.tile([P, KT, DH4], F16)
            nc.sync.dma_start(out=w_t[:], in_=W[:])
            bias_t = const.tile([P, DH4 // P], F32)
            nc.sync.dma_start(out=bias_t[:], in_=bias[:])

            if probe == "megachain":
                # timing probe: one uninterrupted 1024-matmul stream, all
                # accumulating into a single psum bank (garbage math)
                xh_p = const.tile([P, KT, NCH, CHUNK], F16)
                nc.sync.dma_start(out=xh_p[:], in_=xhT[:, :, 0:MACRO])
                loop = tc.For_i(0, loop_n, 1) if loop_n else nullcontext()
                with loop:
                    zp0 = psum.tile([P, CHUNK], F32, tag="mega")
                    nmm = NMACRO * NDC * 4 * KT * NCH
                    for i in range(nmm):
                        ch = i % NCH
                        kc = (i // NCH) % KT
                        nci = (i // (NCH * KT)) % 16
                        nc.tensor.matmul(
                            zp0[:], w_t[:, kc, nci * P:(nci + 1) * P],
                            xh_p[:, kc, ch, :],
                            start=(i == 0), stop=(i == nmm - 1),
                            skip_group_check=True,
                        )
                    dump = work.tile([P, 16], F32, tag="dump")
                    nc.scalar.activation(dump[:], zp0[:, 0:16], AF.Copy)

            loop = (
                tc.For_i(0, loop_n, 1)
                if (loop_n and probe != "megachain")
                else nullcontext()
            )
            with loop:
                # hoist ALL input DMAs to the body start: the SP HWDGE ring
                # then carries only input transfers (output DMAs go on the
                # ACT ring), so prefetch is never head-of-line blocked.
                in_tiles = []
                for mc in range(NMACRO if probe != "megachain" else 0):
                    bsl = slice(mc * MACRO, (mc + 1) * MACRO)
                    xh_t = io.tile([P, KT, NCH, CHUNK], F16, tag=f"xh{mc}",
                                   name=f"xh_t_{mc}", bufs=1)
                    ct_t = io.tile([P, NDC, NCH, CHUNK], F16, tag=f"ct{mc}",
                                   name=f"ct_t_{mc}", bufs=1)
                    if not skip_in_dma:
                        nc.sync.dma_start(out=xh_t[:], in_=xhT[:, :, bsl])
                        nc.sync.dma_start(out=ct_t[:], in_=CT[:, :, bsl])
                    else:
                        # touch the tiles so the tile tracker sees a writer
                        nc.sync.dma_start(
                            out=xh_t[:, 0, 0, 0:16], in_=xhT[:, 0, 0:16]
                        )
                        nc.sync.dma_start(
                            out=ct_t[:, 0, 0, 0:16], in_=CT[:, 0, 0:16]
                        )
                    in_tiles.append((xh_t, ct_t))

                # output DMAs are triggered from the ACT ring, delayed by one
                # d-chunk so the trigger's wait (DVE hn write) is satisfied
                # before ACT reaches it -> no ACT stall, SP ring stays
                # input-only for clean cross-iteration prefetch
                pending_out = []

                def flush_out():
                    while pending_out:
                        dst_cn, src_cn, dst_hn, src_hn = pending_out.pop(0)
                        nc.scalar.dma_start(out=dst_cn, in_=src_cn)
                        nc.scalar.dma_start(out=dst_hn, in_=src_hn)

                for mc in range(NMACRO if probe != "megachain" else 0):
                    bsl = slice(mc * MACRO, (mc + 1) * MACRO)
                    xh_t, ct_t = in_tiles[mc]

                    for dc in range(NDC):
                        flush_out()
                        # 4 gate tiles for this d-chunk, all batch chunks
                        gt4 = [
                            gates.tile(
                                [P, NCH, CHUNK], F16, tag=f"g{g}",
                                name=f"gate{g}_{mc}_{dc}",
                            )
                            for g in range(4)
                        ]
                        for g, fn in enumerate(
                            [AF.Sigmoid, AF.Sigmoid, AF.Sigmoid, AF.Tanh]
                        ):
                            nci = g * NDC + dc
                            nsl = slice(nci * P, (nci + 1) * P)
                            zp = psum.tile([P, NCH, CHUNK], F32, tag="zp")
                            if probe == "nostart":
                                # every matmul start+stop (no accumulation;
                                # garbage results, timing probe only)
                                for kc in range(KT):
                                    lhsT = w_t[:, kc, nsl]
                                    for ch in range(NCH):
                                        nc.tensor.matmul(
                                            zp[:, ch, :], lhsT,
                                            xh_t[:, kc, ch, :],
                                            start=True, stop=True,
                                            skip_group_check=True,
                                        )
                            elif probe == "bankseq":
                                # accumulate one bank to completion before
                                # moving to the next (no lhsT adjacency)
                                for ch in range(NCH):
                                    for kc in range(KT):
                                        nc.tensor.matmul(
                                            zp[:, ch, :], w_t[:, kc, nsl],
                                            xh_t[:, kc, ch, :],
                                            start=(kc == 0),
                                            stop=(kc == KT - 1),
                                        )
                            else:
                                # chunk-outer: finish bank ch's whole chain,
                                # then evacuate it on ACT while the PE fills
                                # bank ch+1 (different banks -> collision-
                                # safe, and chunk-outer accumulation costs
                                # the PE nothing per the bankseq probe)
                                for ch in range(NCH):
                                    for kc in range(KT):
                                        nc.tensor.matmul(
                                            zp[:, ch, :],
                                            w_t[:, kc, nsl],
                                            xh_t[:, kc, ch, :],
                                            start=(kc == 0),
                                            stop=(kc == KT - 1),
                                        )
                                    if not skip_epilogue:
                                        nc.scalar.activation(
                                            gt4[g][:, ch, :], zp[:, ch, :],
                                            fn, bias=bias_t[:, nci:nci + 1],
                                        )
                            if skip_epilogue:
                                # minimal psum release: tiny ACT copy
                                dump = work.tile([P, 16], F32, tag="dump")
                                nc.scalar.activation(
                                    dump[:], zp[:, 0, 0:16], AF.Copy
                                )
                                continue
                        if skip_epilogue:
                            continue
                        it, ft, ot, gg = gt4
                        cn_t = work.tile([P, NCH, CHUNK], F16, tag="cn")
                        hn_t = work.tile([P, NCH, CHUNK], F16, tag="hn")
                        for ch in range(NCH):
                            fc = work.tile([P, CHUNK], F16, tag="fc")
                            nc.vector.tensor_mul(
                                fc[:], ft[:, ch, :], ct_t[:, dc, ch, :]
                            )
                            ig = work.tile([P, CHUNK], F16, tag="ig")
                            nc.vector.tensor_mul(ig[:], it[:, ch, :], gg[:, ch, :])
                            nc.vector.tensor_add(cn_t[:, ch, :], fc[:], ig[:])
                            tch = work.tile([P, CHUNK], F16, tag="tch")
                            nc.scalar.activation(tch[:], cn_t[:, ch, :], AF.Tanh)
                            nc.vector.tensor_mul(
                                hn_t[:, ch, :], ot[:, ch, :], tch[:]
                            )
                        if not skip_out_dma:
                            # per-chunk output DMAs: finer wait granularity
                            # and a 4x smaller final transfer in the body tail
                            for ch in range(NCH):
                                csl = slice(
                                    mc * MACRO + ch * CHUNK,
                                    mc * MACRO + (ch + 1) * CHUNK,
                                )
                                pending_out.append(
                                    (CnT[:, dc, csl], cn_t[:, ch, :],
                                     HnT[:, dc, csl], hn_t[:, ch, :])
                                )
                if probe != "megachain":
                    flush_out()
    fanout_multi_waits(nc)
    if DEDUP_LDWEIGHTS:
        dedup_ldweights(nc)
    if COALESCE_SEM:
        coalesce_mm_sem_updates(nc)
    return nc


_NC = None


def _get_nc():
    global _NC
    if _NC is None:
        _NC = build_nc()
    return _NC


def make_in_maps(x, C, h, Wx, bx, Wh, bh):
    x = np.asarray(x, dtype=np.float32)
    C = np.asarray(C, dtype=np.float32)
    h = np.asarray(h, dtype=np.float32)
    Wfull = np.concatenate(
        [np.asarray(Wx, np.float32), np.asarray(Wh, np.float32)], axis=0
    )
    W_dr = np.ascontiguousarray(
        Wfull.reshape(KT, P, DH4).transpose(1, 0, 2)
    ).astype(NP_F16)
    bias = np.asarray(bx, np.float32) + np.asarray(bh, np.float32)
    bias_dr = np.ascontiguousarray(bias.reshape(DH4 // P, P).T)
    in_maps = []
    for c in range(N_CORES):
        sl = slice(c * B_CORE, (c + 1) * B_CORE)
        xh = np.concatenate([x[sl], h[sl]], axis=1)          # [4096, 1024]
        xhT_dr = np.ascontiguousarray(
            xh.T.reshape(KT, P, B_CORE).transpose(1, 0, 2)
        ).astype(NP_F16)
        CT_dr = np.ascontiguousarray(
            C[sl].T.reshape(NDC, P, B_CORE).transpose(1, 0, 2)
        ).astype(NP_F16)
        in_maps.append(
            {"xhT": xhT_dr, "CT": CT_dr, "W": W_dr, "bias": bias_dr}
        )
    return in_maps


def _untranspose(arr):
    # [p, dc, b] -> [b, dc*128+p]
    return (
        np.asarray(arr).astype(np.float32).transpose(2, 1, 0).reshape(B_CORE, DH)
    )


def kernel(x, C, h, Wx, bx, Wh, bh):
    nc = _get_nc()
    in_maps = make_in_maps(x, C, h, Wx, bx, Wh, bh)
    res = run_bass_kernel_spmd(nc, in_maps, list(range(N_CORES)))
    C_new = np.concatenate(
        [_untranspose(res.results[c]["CnT"]) for c in range(N_CORES)], axis=0
    )
    h_new = np.concatenate(
        [_untranspose(res.results[c]["HnT"]) for c in range(N_CORES)], axis=0
    )
    return (C_new, h_new)
